# revision 16
# baseline (speedup 1.0000x reference)
"""BallQueryAttention TRN2 kernel.

Math: reference computes softmax over a binary ball mask (d2 <= R^2), then
mask-softmax @ x.  exp of a 0/1 mask takes only values {1, e}, so

  out[i] = (S + (e-1) * sum_{j in ball(i)} x_j) / (N + (e-1) * cnt_i)

with S = colsum(x).  Sharding: rows (i) across 8 cores, x replicated.

Per core (row shard of 1024):
  - Gram tiles Gt[j_tile(128 part), i(1024 free)] via 2 fp16 hi/lo-split
    matmuls (fp32-class accuracy on the distance threshold).  The sq_i term
    rides 3 fp16-split augmentation rows on the moving operand; the sq_j
    term rides fp32 per-partition bias/threshold in the compare op.
  - mask compare split across Vector (is_ge -> {0,2}) and Scalar
    (Sign -> {-1,1}) engines, writing fp16 masks.
  - accumulating [x|1]^T @ mask matmul -> [65, 1024] in PSUM, plus
    ones-column sums (SALL over all tiles, SAO over sign tiles) to undo
    the {0,2}/{-1,1} conventions:
      numer/denom = SALL + K1*(OUT2 + SAO),  K1 = (e-1)/2
  - PE transpose + reciprocal + per-partition scale for the final divide.

Transposed fp16 layouts (d-on-partitions) are produced by DMA-transpose of
[rows, 128] fp16 DRAM scratch ([hi|lo] packed), pipelined in 8 column
groups so the PE starts after ~1/8 of the preamble.
"""

import sys

sys.path.insert(0, "/opt/trn_rl_repo")

import numpy as np

import concourse.bass as bass
import concourse.tile as tile
from concourse import bacc, masks, mybir
from concourse.bass_utils import run_bass_kernel_spmd

F32 = mybir.dt.float32
F16 = mybir.dt.float16
AF = mybir.ActivationFunctionType
OP = mybir.AluOpType

N = 8192
D = 64
NCORES = 8
ROWS = N // NCORES          # 1024 rows per core
JT = N // 128               # 64 j-tiles
IT = ROWS // 128            # 8 i-tiles
NG = 16                     # preamble column groups
TPG = JT // NG              # j-tiles per group
R2 = 11.0 * 11.0
K1 = (np.e - 1.0) / 2.0


def _body(nc, tc, pools, xf, xi, outd, dram):
    const, scratch, gpool, mpool, apool, spool = pools
    ts = bass.ts

    # ---------------- persistent tiles ----------------
    W1g = [const.tile([128, TPG * 128], F16, name=f"W1_{g}", tag=f"W1_{g}")
           for g in range(NG)]
    W2g = [const.tile([128, TPG * 128], F16, name=f"W2_{g}", tag=f"W2_{g}")
           for g in range(NG)]
    XW = const.tile([128, 65 * JT], F16, tag="XW")
    R1 = const.tile([128, ROWS], F16, tag="R1")     # [hiT_i; hiT_i]
    R2t = const.tile([128, ROWS], F16, tag="R2")    # [loT_i; v1; v2; v3; 0]
    biasA = const.tile([128, JT], F32, tag="biasA")
    thrD = const.tile([128, JT], F32, tag="thrD")
    ONEC = const.tile([128, 1], F16, tag="ONEC")
    IDN = const.tile([65, 65], F32, tag="IDN")

    # ---------------- preamble: i side (small) ----------------
    r1d = dram.tile([ROWS, 128], F16, tag="r1d")
    r2d = dram.tile([ROWS, 128], F16, tag="r2d")

    xitp = scratch.tile([128, IT * D], F32, tag="xitp")  # row p*IT + t
    nc.scalar.dma_start(xitp[:], xi.rearrange("(p t) d -> p (t d)", p=128))
    xitp3 = xitp[:].rearrange("p (t d) -> p t d", d=D)

    r1sb = scratch.tile([128, IT * 128], F16, tag="r1sb")
    r13 = r1sb[:].rearrange("p (t e) -> p t e", e=128)
    nc.vector.tensor_copy(r13[:, :, 0:D], xitp3)          # hi_i
    r2sb = scratch.tile([128, IT * 128], F16, tag="r2sb")
    r23 = r2sb[:].rearrange("p (t e) -> p t e", e=128)
    nc.gpsimd.memset(r23[:, :, D:128], 0.0)
    nc.vector.tensor_tensor(r23[:, :, 0:D], xitp3, r13[:, :, 0:D], OP.subtract)  # lo_i
    nc.vector.tensor_copy(r13[:, :, D : 2 * D], r13[:, :, 0:D])  # dup hi_i

    s2i = scratch.tile([128, IT * D], F32, tag="s2i")
    nc.scalar.activation(s2i[:], xitp[:], AF.Square)
    sqit = scratch.tile([128, IT], F32, tag="sqit")
    nc.vector.tensor_reduce(sqit[:], s2i[:].rearrange("p (t d) -> p t d", d=D),
                            axis=mybir.AxisListType.X, op=OP.add)
    vfull = scratch.tile([128, IT], F32, tag="vfull")
    nc.vector.tensor_scalar(vfull[:], sqit[:], -0.5, None, OP.mult)
    v1 = scratch.tile([128, IT], F16, tag="v1")
    nc.vector.tensor_copy(v1[:], vfull[:])
    rv1 = scratch.tile([128, IT], F32, tag="rv1")
    nc.vector.tensor_tensor(rv1[:], vfull[:], v1[:], OP.subtract)
    v2 = scratch.tile([128, IT], F16, tag="v2")
    nc.vector.tensor_copy(v2[:], rv1[:])
    rv2 = scratch.tile([128, IT], F32, tag="rv2")
    nc.vector.tensor_tensor(rv2[:], rv1[:], v2[:], OP.subtract)
    v3 = scratch.tile([128, IT], F16, tag="v3")
    nc.vector.tensor_copy(v3[:], rv2[:])
    for k, vk in enumerate((v1, v2, v3)):
        nc.vector.tensor_copy(
            r23[:, :, D + k : D + k + 1],
            vk[:].rearrange("p (t u) -> p t u", u=1),
        )
    nc.scalar.dma_start(r1d[:].rearrange("(p t) e -> p (t e)", p=128), r1sb[:])
    nc.scalar.dma_start(r2d[:].rearrange("(p t) e -> p (t e)", p=128), r2sb[:])
    nc.scalar.dma_start(R1[:], r1d[:], transpose=True)
    nc.scalar.dma_start(R2t[:], r2d[:], transpose=True)

    nc.vector.memset(ONEC[:], 1.0)
    masks.make_identity(nc, IDN[:])

    # trigger the Sign act-table load early so it overlaps the preamble
    dumm = spool.tile([128, 1], F32, tag="dumm")
    nc.scalar.activation(dumm[:], xitp[:, 0:1], AF.Sign)

    # ---------------- preamble: j side, pipelined in NG groups ----------
    for g in range(NG):
        rows = TPG * 128  # rows covered by this group
        xtp = scratch.tile([128, TPG * D], F32, tag="xtp")  # bufs>1 pool
        nc.sync.dma_start(
            xtp[:].rearrange("p (t d) -> p t d", d=D),
            xf.rearrange("(t p) d -> p t d", p=128)[:, g * TPG : (g + 1) * TPG, :],
        )
        xtp3 = xtp[:].rearrange("p (t d) -> p t d", d=D)

        # XW slice for this group
        xw3 = XW[:].rearrange("p (t e) -> p t e", e=65)[:, g * TPG : (g + 1) * TPG, :]
        nc.vector.tensor_copy(xw3[:, :, 0:D], xtp3)
        nc.gpsimd.memset(xw3[:, :, D : D + 1], 1.0)

        # hi/lo staging -> hilod_g -> two transposes
        hilo = scratch.tile([128, TPG * 128], F16, tag="hilo")
        hl3 = hilo[:].rearrange("p (t e) -> p t e", e=128)
        nc.vector.tensor_copy(hl3[:, :, 0:D], xtp3)
        nc.vector.tensor_tensor(hl3[:, :, D : 2 * D], xtp3, hl3[:, :, 0:D],
                                OP.subtract)
        hilod = dram.tile([rows, 128], F16, tag="hilod")
        nc.scalar.dma_start(hilod[:].rearrange("(t p) e -> p t e", p=128), hl3)
        nc.sync.dma_start(W1g[g][:], hilod[:], transpose=True)
        nc.sync.dma_start(W2g[g][:], hilod[:], transpose=True)
        nc.gpsimd.memset(W2g[g][D : D + 3, :], 1.0)  # ones rows for aug

        # sq_j -> bias/thr columns for this group's tiles
        s2 = scratch.tile([128, TPG * D], F32, tag="s2")
        nc.scalar.activation(s2[:], xtp[:], AF.Square)
        sl = slice(g * TPG, (g + 1) * TPG)
        nc.vector.tensor_reduce(biasA[:, sl],
                                s2[:].rearrange("p (t d) -> p t d", d=D),
                                axis=mybir.AxisListType.X, op=OP.add)
        nc.vector.tensor_scalar(thrD[:, sl], biasA[:, sl], 0.5, -R2 / 2.0,
                                OP.mult, OP.add)
        nc.vector.tensor_scalar(biasA[:, sl], biasA[:, sl], -0.5, R2 / 2.0,
                                OP.mult, OP.add)

    # ---------------- psum accumulators ----------------
    # column block 0:512 always gets the DVE {0,2} mask convention and
    # block 512:1024 the ACT {-1,1} one, so the sign-correction term is
    # just SALL itself:
    #   P[:, 0:512]    = K1*OUT2 + SALL
    #   P[:, 512:1024] = K1*OUT2 + (1+K1)*SALL
    OUT2 = apool.tile([65, ROWS], F32, tag="OUT2")
    SALL = apool.tile([65, 1], F32, tag="SALL")

    # ------- main loop over half j-tiles, pass C lagged by 2 halves ------
    LAG = 2
    NH = 2 * JT
    mks = {}
    for idx in range(NH + LAG):
        if idx < NH:
            t, h = divmod(idx, 2)
            g, tt = divmod(t, TPG)
            cs = slice(512 * h, 512 * (h + 1))
            Gh = gpool.tile([128, 512], F32, tag="G")
            nc.tensor.matmul(Gh[:], W1g[g][:, ts(tt, 128)], R1[:, cs],
                             start=True, stop=False)
            nc.tensor.matmul(Gh[:], W2g[g][0:67, ts(tt, 128)],
                             R2t[0:67, cs], start=False, stop=True)
            mk = mpool.tile([128, 512], F16, tag="mk")
            if idx % 2 == 0:
                nc.vector.tensor_scalar(mk[:], Gh[:], thrD[:, t : t + 1], 2.0,
                                        OP.is_ge, OP.mult)
            else:
                nc.scalar.activation(mk[:], Gh[:], AF.Sign,
                                     bias=biasA[:, t : t + 1])
            mks[idx] = mk
        if idx >= LAG:
            jdx = idx - LAG
            t, h = divmod(jdx, 2)
            cs = slice(512 * h, 512 * (h + 1))
            xws = XW[:, 65 * t : 65 * (t + 1)]
            nc.tensor.matmul(OUT2[:, cs], xws, mks.pop(jdx)[:],
                             start=(t == 0), stop=(t == JT - 1))
            if h == 1:
                nc.tensor.matmul(SALL[:], xws, ONEC[:],
                                 start=(t == 0), stop=(t == JT - 1))

    # ---------------- tail (per i-chunk, DVE/ACT alternating) -----------
    sallsb = spool.tile([65, 1], F32, tag="sallsb")
    nc.vector.tensor_copy(sallsb[:], SALL[:])
    b1sb = spool.tile([65, 1], F32, tag="b1sb")
    nc.vector.tensor_scalar(b1sb[:], sallsb[:], 1.0 + K1, None, OP.mult)

    for c in range(IT):
        bap = sallsb if c < IT // 2 else b1sb
        pc = spool.tile([65, 128], F32, tag="pc")
        if c % 2 == 0:
            nc.vector.tensor_scalar(pc[:], OUT2[:, ts(c, 128)], K1, bap[:],
                                    OP.mult, OP.add)
        else:
            nc.scalar.activation(pc[:], OUT2[:, ts(c, 128)], AF.Identity,
                                 bias=bap[:], scale=K1)
        pt = gpool.tile([128, 65], F32, tag="G")
        nc.tensor.transpose(pt[:], pc[:], IDN[:])
        dinv = spool.tile([128, 1], F32, tag="dinv")
        nc.vector.reciprocal(dinv[:], pt[:, D : D + 1])
        ot = spool.tile([128, D], F32, tag="ot")
        nc.vector.tensor_scalar(ot[:], pt[:, 0:D], dinv[:], None, OP.mult)
        nc.sync.dma_start(outd[ts(c, 128), :], ot[:])


def build_module(loop_n=1):
    nc = bacc.Bacc("TRN2", target_bir_lowering=False, debug=False,
                   num_devices=NCORES)
    xf_d = nc.dram_tensor("xf", [N, D], F32, kind="ExternalInput")
    xi_d = nc.dram_tensor("xi", [ROWS, D], F32, kind="ExternalInput")
    out_d = nc.dram_tensor("out", [ROWS, D], F32, kind="ExternalOutput")

    with tile.TileContext(nc) as tc:
        with (
            tc.tile_pool(name="const", bufs=1) as const,
            tc.tile_pool(name="scratch", bufs=2) as scratch,
            tc.tile_pool(name="gpool", bufs=5, space="PSUM") as gpool,
            tc.tile_pool(name="acc", bufs=1, space="PSUM") as apool,
            tc.tile_pool(name="mk", bufs=4) as mpool,
            tc.tile_pool(name="small", bufs=3) as spool,
            tc.tile_pool(name="dram", bufs=3, space="DRAM") as dram,
        ):
            pools = (const, scratch, gpool, mpool, apool, spool)
            if loop_n == 1:
                _body(nc, tc, pools, xf_d.ap(), xi_d.ap(), out_d.ap(), dram)
            else:
                with tc.For_i(0, loop_n) as _:
                    _body(nc, tc, pools, xf_d.ap(), xi_d.ap(), out_d.ap(), dram)
    nc.finalize()
    return nc


_module_cache = {}


def _get_module(loop_n=1):
    if loop_n not in _module_cache:
        _module_cache[loop_n] = build_module(loop_n)
    return _module_cache[loop_n]


def kernel(x, adj=None):
    x = np.ascontiguousarray(np.asarray(x, dtype=np.float32))
    assert x.shape == (N, D)
    nc = _get_module(1)
    in_maps = [
        {"xf": x, "xi": x[c * ROWS : (c + 1) * ROWS]} for c in range(NCORES)
    ]
    res = run_bass_kernel_spmd(nc, in_maps, core_ids=list(range(NCORES)))
    return np.concatenate([res.results[c]["out"] for c in range(NCORES)], axis=0)
